# revision 45
# baseline (speedup 1.0000x reference)
"""MPNN (2x NNConv + BN + global mean pool + MLP) on 8 Trainium2 cores.

Strategy (node-sharded message passing), v2:
  * Never materialize We=[E,in_c,out_c].  msg[e] = (z[e] (x) xs[e]) @ W2r
    where z=relu(ea@W1+b1), xs=x[src], W2r = reshape of W2.  Since the
    segment-sum over dst commutes with the (shared) @W2r, we scatter the
    per-edge outer products u[e]=(z (x) xs_scaled) into per-node U first,
    then do ONE matmul per node tile:  agg = U @ W2r.
  * Nodes are bin-packed into 128-node windows balanced by edge count;
    each core owns 10 windows.  Edges are grouped per window (padded to
    T tiles of 128).  Scatter = one-hot matmul on the PE accumulating
    U^T chunks in PSUM, pipelined per 128-wide chunk with the
    PSUM->SBUF cast (scalar engine) and the node matmul.
  * v2 layout changes vs v1: all per-edge tables are preloaded once as
    resident SBUF tiles (no per-tile DMAs); x[src] for layer 1 is
    gathered host-side (pure indexing) so layer 1 has no indirect DMAs;
    bias1/bias2 ride as an extra ones-row on the root operand; BN1 is
    folded into the layer-2 gather; BN2+pool commute so BN2 is applied
    after the final AllReduce.  Collectives use Shared outputs; the
    layer-2 edge-MLP z tiles / one-hots / h1^T transposes are computed
    during the h1 AllGather.
"""

import sys

import numpy as np

try:
    import concourse.bass as bass  # noqa
except Exception:  # pragma: no cover
    sys.path.insert(0, "/opt/trn_rl_repo")

import ml_dtypes
import concourse.bacc as bacc
import concourse.bass as bass
import concourse.mybir as mybir
import concourse.tile as tile
from concourse.bass import IndirectOffsetOnAxis
from concourse.bass_utils import run_bass_kernel_spmd
from concourse.masks import make_identity

P = 128
NCORES = 8
N = 10000
E = 30000
NG = 256
IN_C = 16
EDGE_C = 8
KH = 32  # edge-MLP hidden width
H1 = 64  # conv1 out channels
H2 = 128  # conv2 out channels
WPC = 10  # windows per core
NPADC = WPC * P  # padded nodes per core (1280)
NSTR = NPADC + 4  # h1 slice rows per core incl. 4 BN-stats rows (f32 bits in bf16 pairs)
EPS = 1e-5
f32 = mybir.dt.float32
bf16 = mybir.dt.bfloat16
i32 = mybir.dt.int32

U1 = KH * IN_C  # 512
U1E = 640  # padded to 5 chunks of 128 (u | xs | zeros)
U2 = KH * H1  # 2048
U2E = 2176  # padded to 17 chunks of 128 (u | xs | zeros)


# --------------------------------------------------------------------------
# host-side preprocessing: index/layout work only
# --------------------------------------------------------------------------
def _preprocess(x, edge_index, edge_attr, batch):
    import heapq

    src = np.asarray(edge_index[0], dtype=np.int64)
    dst = np.asarray(edge_index[1], dtype=np.int64)
    deg = np.bincount(dst, minlength=N).astype(np.int64)

    # ---- bin-pack nodes into NCORES*WPC windows of exactly <=128 nodes,
    # balancing per-window edge counts (LPT greedy) ----
    NW = NCORES * WPC
    order = np.argsort(-deg, kind="stable")
    wsum = np.zeros(NW, dtype=np.int64)
    wcnt = np.zeros(NW, dtype=np.int64)
    win_of = np.empty(N, dtype=np.int64)
    slot_of = np.empty(N, dtype=np.int64)
    heap = [(0, w) for w in range(NW)]
    heapq.heapify(heap)
    for n in order:
        while True:
            _, w = heapq.heappop(heap)
            if wcnt[w] < P:
                break
        win_of[n] = w
        slot_of[n] = wcnt[w]
        wcnt[w] += 1
        wsum[w] += deg[n]
        if wcnt[w] < P:
            heapq.heappush(heap, (int(wsum[w]), w))

    T = max(1, int(-(-int(wsum.max()) // P)))  # tiles (of 128 edges) per window
    NT = WPC * T  # edge tiles per core
    ES = NT * P  # edge slots per core

    core_of = win_of // WPC
    lpos = (win_of % WPC) * P + slot_of

    # ---- per-edge placement ----
    ew = win_of[dst]
    eorder = np.argsort(ew, kind="stable")
    inv_cnt = 1.0 / np.maximum(deg, 1).astype(np.float32)

    ea_s = np.zeros((NCORES, ES, EDGE_C), dtype=np.float32)
    ones_s = np.zeros((NCORES, ES), dtype=np.float32)
    srcx_s = np.zeros((NCORES, ES), dtype=np.int64)
    srch_s = np.zeros((NCORES, ES), dtype=np.int32)
    dstrel_s = np.full((NCORES, ES), -1.0, dtype=np.float32)
    icnt_s = np.zeros((NCORES, ES), dtype=np.float32)

    ew_sorted = ew[eorder]
    starts = np.searchsorted(ew_sorted, np.arange(NW))
    ends = np.searchsorted(ew_sorted, np.arange(NW) + 1)
    HA = 5 * P  # rows in the first AllGather half
    HB = NSTR - HA
    SENT = 1 << 20
    srchA_s = np.zeros((NCORES, ES), dtype=np.int32)
    srchB_s = np.zeros((NCORES, ES), dtype=np.int32)
    isA_s = np.zeros((NCORES, ES), dtype=bool)
    isB_s = np.zeros((NCORES, ES), dtype=bool)
    ea_np = np.asarray(edge_attr, dtype=np.float32)
    for w in range(NW):
        es = eorder[starts[w] : ends[w]]
        # sort the window's edges: sources in half A first, then half B
        eA = lpos[src[es]] < HA
        es = es[np.argsort(~eA, kind="stable")]
        c = w // WPC
        base = (w % WPC) * T * P
        k = len(es)
        assert k <= T * P
        sl = slice(base, base + k)
        ea_s[c, sl] = ea_np[es]
        ones_s[c, sl] = 1.0
        srcx_s[c, sl] = src[es]
        _lp = lpos[src[es]]
        _co = core_of[src[es]]
        _inA = _lp < HA
        srch_s[c, sl] = np.where(
            _inA, _co * HA + _lp, NCORES * HA + _co * HB + (_lp - HA)
        ).astype(np.int32)
        srchA_s[c, sl] = np.where(_inA, _co * HA + _lp, SENT).astype(np.int32)
        srchB_s[c, sl] = np.where(~_inA, _co * HB + (_lp - HA), SENT).astype(np.int32)
        isA_s[c, sl] = _inA
        isB_s[c, sl] = ~_inA
        dstrel_s[c, sl] = slot_of[dst[es]]
        icnt_s[c, sl] = inv_cnt[dst[es]]
    # pad slots: valid row 0 in both tables (values are zeroed by icnt=0)
    pad = ~(isA_s | isB_s)
    srchA_s[pad] = 0
    srchB_s[pad] = 0
    # per-tile gather type, unioned across cores (the program is SPMD):
    # 0 = pure-A, 1 = pure-B, 2 = mixed
    hasA = isA_s.reshape(NCORES, NT, P).any(axis=(0, 2))
    hasB = isB_s.reshape(NCORES, NT, P).any(axis=(0, 2))
    ttypes = tuple(
        (2 if (a and b) else (1 if b else 0)) for a, b in zip(hasA, hasB)
    )

    eaT = np.concatenate(
        [np.transpose(ea_s, (0, 2, 1)), ones_s[:, None, :]], axis=1
    ).astype(np.float32)

    # host gather of x[src] (pure indexing), tile-major [P, NT*IN_C]
    x_np = np.asarray(x, dtype=np.float32)
    xsrc = x_np[srcx_s.reshape(NCORES, NT, P)]  # [C, NT, P, IN_C]
    xsrc_s = np.ascontiguousarray(
        xsrc.transpose(0, 2, 1, 3).reshape(NCORES, P, NT * IN_C)
    )

    # per-edge tables in [P(slot-in-tile), NT] layout
    def t_major(a):
        return np.ascontiguousarray(
            a.reshape(NCORES, NT, P).transpose(0, 2, 1)
        )

    srch_t = t_major(srch_s)
    srchA_t = t_major(srchA_s)
    srchB_t = t_major(srchB_s)
    drel_t = t_major(dstrel_s)
    icnt_t = t_major(icnt_s)
    ohall = (drel_t[:, :, :, None] == np.arange(P, dtype=np.float32)).astype(
        ml_dtypes.bfloat16
    ).reshape(NCORES, P, NT * P)

    # ---- per-node per-core tables ----
    batch = np.asarray(batch, dtype=np.int64)
    gcnt = np.bincount(batch, minlength=NG).astype(np.int64)
    igc_node = (1.0 / np.maximum(gcnt, 1).astype(np.float32))[batch]
    gmask = (gcnt > 0).astype(np.float32)

    xTa_s = np.zeros((NCORES, IN_C + 1, NPADC), dtype=np.float32)
    xTa_s[:, IN_C, :] = 1.0  # ones row for fused bias
    batch_s = np.full((NCORES, P, WPC), -1.0, dtype=np.float32)
    igc_s = np.zeros((NCORES, P, WPC), dtype=np.float32)
    vmask_s = np.zeros((NCORES, P, WPC), dtype=np.float32)
    for c in range(NCORES):
        m = core_of == c
        lp = lpos[m]
        xTa_s[c][:IN_C, lp] = x_np[m].T
        batch_s[c][lp % P, lp // P] = batch[m].astype(np.float32)
        igc_s[c][lp % P, lp // P] = igc_node[m]
        vmask_s[c][lp % P, lp // P] = 1.0

    ohgall = (batch_s[:, :, :, None] == np.arange(NG, dtype=np.float32)).astype(
        ml_dtypes.bfloat16
    ).reshape(NCORES, P, WPC * NG)

    return dict(
        T=T, ES=ES, NT=NT, eaT=eaT, srch=srch_t, srchA=srchA_t, srchB=srchB_t,
        ttypes=ttypes, drel=drel_t, icnt=icnt_t,
        xsrc=xsrc_s, xTa=xTa_s, batchrel=batch_s, igc=igc_s, vmask=vmask_s,
        gmask=gmask, ohall=ohall, ohgall=ohgall,
    )


def _weights(p):
    w = {}
    w["W1a1"] = np.concatenate([p["nn1_W1"], p["nn1_b1"][None, :]], 0).astype(np.float32)
    w["W1a2"] = np.concatenate([p["nn2_W1"], p["nn2_b1"][None, :]], 0).astype(np.float32)
    wp1 = np.zeros((U1E, H1), dtype=np.float32)
    wp1[:U1] = p["nn1_W2"].reshape(KH, IN_C, H1).reshape(U1, H1)
    wp1[U1 : U1 + IN_C] = p["nn1_b2"].reshape(IN_C, H1)
    w["Wp1"] = wp1
    wp2 = np.zeros((U2E, H2), dtype=np.float32)
    wp2[:U2] = p["nn2_W2"].reshape(KH, H1, H2).reshape(U2, H2)
    wp2[U2 : U2 + H1] = p["nn2_b2"].reshape(H1, H2)
    w["Wp2"] = wp2
    w["root1a"] = np.concatenate(
        [np.asarray(p["root1"], np.float32), np.asarray(p["bias1"], np.float32)[None, :]], 0
    )
    w["root2a"] = np.concatenate(
        [np.asarray(p["root2"], np.float32), np.asarray(p["bias2"], np.float32)[None, :]], 0
    )
    w["bng1r"] = np.asarray(p["bn1_g"], np.float32)[None, :]
    w["bnb1r"] = np.asarray(p["bn1_b"], np.float32)[None, :]
    w["bng2"] = np.asarray(p["bn2_g"], np.float32)[:, None]
    w["bnb2"] = np.asarray(p["bn2_b"], np.float32)[:, None]
    w["l1W"] = np.asarray(p["lin1_W"], np.float32)
    w["l1b"] = np.asarray(p["lin1_b"], np.float32)[:, None]
    w["l2W"] = np.asarray(p["lin2_W"], np.float32)
    w["l2b"] = np.asarray(p["lin2_b"], np.float32)[None, :]
    w["iota128"] = np.broadcast_to(np.arange(P, dtype=np.float32), (P, P)).copy()
    w["iota256"] = np.broadcast_to(np.arange(NG, dtype=np.float32), (P, NG)).copy()
    w["onesP"] = np.ones((P, 1), dtype=np.float32)
    w["onesr"] = np.ones((1, P), dtype=np.float32)
    return w


# --------------------------------------------------------------------------
# device program (identical for all cores; per-core data comes via inputs)
# --------------------------------------------------------------------------
def build_program(T, ES, ttypes):
    NT = WPC * T
    HA = 5 * P
    HB = NSTR - HA
    AL = mybir.AluOpType
    AF = mybir.ActivationFunctionType
    nc = bacc.Bacc("TRN2", target_bir_lowering=False, debug=False, num_devices=NCORES)

    def din(name, shape, dtype=f32):
        return nc.dram_tensor(name, shape, dtype, kind="ExternalInput").ap()

    eaT_d = din("eaT", [EDGE_C + 1, ES], bf16)
    srchA_d = din("srchA", [P, NT], i32)
    srchB_d = din("srchB", [P, NT], i32)
    ohall_d = din("ohall", [P, NT * P], bf16)
    ohgall_d = din("ohgall", [P, WPC * NG], bf16)
    icnt_d = din("icnt", [P, NT])
    xsrc_d = din("xsrc", [P, NT * IN_C])
    xTa_d = din("xTa", [IN_C + 1, NPADC], bf16)
    igc_d = din("igc", [P, WPC])
    vmask_d = din("vmask", [P, WPC])
    W1a1_d = din("W1a1", [EDGE_C + 1, KH], bf16)
    W1a2_d = din("W1a2", [EDGE_C + 1, KH], bf16)
    Wp1_d = din("Wp1", [U1E, H1], bf16)
    Wp2_d = din("Wp2", [U2E, H2], bf16)
    root1a_d = din("root1a", [IN_C + 1, H1], bf16)
    root2a_d = din("root2a", [H1 + 1, H2], bf16)
    bng1r_d = din("bng1r", [1, H1])
    bnb1r_d = din("bnb1r", [1, H1])
    bng2_d = din("bng2", [H2, 1])
    bnb2_d = din("bnb2", [H2, 1])
    l1W_d = din("l1W", [H2, H1])
    l1b_d = din("l1b", [H1, 1])
    l2W_d = din("l2W", [H1, 1])
    l2b_d = din("l2b", [1, 1])
    onesP_d = din("onesP", [P, 1])
    onesr_d = din("onesr", [1, P])
    gmaskb_d = din("gmaskb", [P, NG])
    out_d = nc.dram_tensor("out", [1, NG], f32, kind="ExternalOutput").ap()

    NC1 = U1E // P  # 5 chunks
    NC2 = U2E // P  # 17 chunks

    from contextlib import ExitStack

    with tile.TileContext(nc) as tc, ExitStack() as pools:
        cst = pools.enter_context(tc.tile_pool(name="cst", bufs=1))
        sb = pools.enter_context(tc.tile_pool(name="sb", bufs=3))
        stash = pools.enter_context(tc.tile_pool(name="stash", bufs=WPC))
        pp_u = pools.enter_context(tc.tile_pool(name="pp_u", bufs=1, space="PSUM"))
        pp_pre = pools.enter_context(tc.tile_pool(name="pp_pre", bufs=1, space="PSUM"))
        pp_z = pools.enter_context(tc.tile_pool(name="pp_z", bufs=1, space="PSUM"))
        pp_m = pools.enter_context(tc.tile_pool(name="pp_m", bufs=2, space="PSUM"))
        pp_g = pools.enter_context(tc.tile_pool(name="pp_g", bufs=1, space="PSUM"))
        dram = pools.enter_context(tc.tile_pool(name="dram", bufs=1, space="DRAM"))

        # ---- resident constants (spread initial DMAs across engine queues) ----
        _eng = [nc.sync, nc.gpsimd]
        _ei = [0]

        def load(shape, ap, name, dt=f32):
            t = cst.tile(shape, dt, tag=name, name=name)
            _eng[_ei[0] % len(_eng)].dma_start(out=t[:], in_=ap)
            _ei[0] += 1
            return t

        groups = [list(range(NCORES))]
        warm_in = dram.tile([1, 1], f32, tag="warmi")
        warm_out = dram.tile([NCORES, 1], f32, tag="warmo", addr_space="Shared")
        nc.sync.dma_start(out=warm_in[:], in_=onesP_d[:1, :])
        nc.gpsimd.collective_compute(
            "AllGather", mybir.AluOpType.bypass, replica_groups=groups,
            ins=[warm_in.opt()], outs=[warm_out.opt()],
        )
        ident = cst.tile([P, P], f32, tag="ident")
        make_identity(nc, ident[:])
        identb = cst.tile([P, P], bf16, tag="identb")
        make_identity(nc, identb[:])
        # L1-critical tables first so windows can start ASAP
        eaT = load([EDGE_C + 1, ES], eaT_d[:], "eaT", bf16)
        W1a1 = load([EDGE_C + 1, KH], W1a1_d[:], "W1a1", bf16)
        icnt = load([P, NT], icnt_d[:], "icnt")
        xsrc = load([P, NT * IN_C], xsrc_d[:], "xsrc")
        Wp1 = cst.tile([P, NC1, H1], bf16, tag="Wp1")
        nc.gpsimd.dma_start(out=Wp1[:], in_=Wp1_d.rearrange("(c p) o -> p c o", p=P))
        ohall = load([P, NT * P], ohall_d[:], "ohall", bf16)
        xTa = load([IN_C + 1, NPADC], xTa_d[:], "xTa", bf16)
        root1a = load([IN_C + 1, H1], root1a_d[:], "root1a", bf16)
        vmask = load([P, WPC], vmask_d[:], "vmask")
        onesP = load([P, 1], onesP_d[:], "onesP")
        # L2 / tail tables
        W1a2 = load([EDGE_C + 1, KH], W1a2_d[:], "W1a2", bf16)
        Wp2 = cst.tile([P, NC2, H2], bf16, tag="Wp2")
        nc.scalar.dma_start(out=Wp2[:], in_=Wp2_d.rearrange("(c p) o -> p c o", p=P))
        srchA = load([P, NT], srchA_d[:], "srchA", i32)
        srchB = load([P, NT], srchB_d[:], "srchB", i32)
        ohgall = load([P, WPC * NG], ohgall_d[:], "ohgall", bf16)
        root2a = load([H1 + 1, H2], root2a_d[:], "root2a", bf16)
        bng1r = load([1, H1], bng1r_d[:], "bng1r")
        bnb1r = load([1, H1], bnb1r_d[:], "bnb1r")
        bng2 = load([H2, 1], bng2_d[:], "bng2")
        bnb2 = load([H2, 1], bnb2_d[:], "bnb2")
        l1W = load([H2, H1], l1W_d[:], "l1W")
        l1b = load([H1, 1], l1b_d[:], "l1b")
        l2W = load([H1, 1], l2W_d[:], "l2W")
        l2b = load([1, 1], l2b_d[:], "l2b")
        onesPb = cst.tile([P, 1], bf16, tag="onesPb")
        nc.vector.tensor_copy(out=onesPb[:], in_=onesP[:])
        onesr = load([1, P], onesr_d[:], "onesr")
        gmaskb = load([P, NG], gmaskb_d[:], "gmaskb")
        igc = load([P, WPC], igc_d[:], "igc")


        h1_slice = dram.tile([NSTR, H1], bf16, tag="h1s")
        h1A = dram.tile([NCORES * HA, H1], bf16, tag="h1a", addr_space="Shared")
        h1B = dram.tile([NCORES * HB, H1], bf16, tag="h1b", addr_space="Shared")
        finA_loc = dram.tile([P, NG], f32, tag="final")
        finA_g = dram.tile([P, NG], f32, tag="finag", addr_space="Shared")
        fin_loc = dram.tile([P, NG + 2], f32, tag="finl")
        fin_g = dram.tile([P, NG + 2], f32, tag="fing", addr_space="Shared")

        gTp = [None]

        # stash for precomputed per-tile z (bf16)
        z2_all = cst.tile([P, NT, KH], bf16, tag="z2all")

        # u slabs: layer 1 keeps ALL tiles resident (full prepass); layer 2
        # rotates 6 slots.  Zero tails initialized ONCE.
        u1_slab = cst.tile([P, NT, U1E], bf16, tag="u1slab")
        u2_slab = cst.tile([P, 8, U2E], bf16, tag="u2slab")
        for si in range(NT):
            nc.gpsimd.memset(u1_slab[:, si, U1 + IN_C :], 0.0)
        for si in range(8):
            nc.gpsimd.memset(u2_slab[:, si, U2 + H1 :], 0.0)

        # ---- L1 prepass: z, xss and outer products for ALL tiles ----
        ZP1 = pp_z.tile([P, 4, KH], f32, tag="z")
        for t in range(NT):
            s0 = t * P
            zp = ZP1[:, t % 4, :]
            nc.tensor.matmul(
                out=zp[:], lhsT=eaT[:, s0 : s0 + P], rhs=W1a1[:],
                start=True, stop=True,
            )
            z = sb.tile([P, KH], bf16, tag="z_sb", bufs=4)
            nc.vector.tensor_scalar_max(out=z[:], in0=zp[:], scalar1=0.0)
            u = u1_slab[:, t, :]
            xss = u[:, U1 : U1 + IN_C]
            nc.vector.tensor_scalar_mul(
                out=xss, in0=xsrc[:, t * IN_C : (t + 1) * IN_C],
                scalar1=icnt[:, t : t + 1],
            )
            nc.vector.tensor_tensor(
                out=u[:, :U1].rearrange("p (k i) -> p k i", k=KH),
                in0=z.unsqueeze(2).to_broadcast([P, KH, IN_C]),
                in1=xss.unsqueeze(1).to_broadcast([P, KH, IN_C]),
                op=AL.mult,
            )

        # ================= generic conv layer =================
        def conv_layer(layer, scale_bc=None, shift_bc=None, h1T_list=None):
            if layer == 1:
                inc, outc, UE, NCH, W1a, Wp = IN_C, H1, U1E, NC1, W1a1, Wp1
                uw = U1
                STPa = pp_m.tile([1, H1], f32, tag="m", name="STP1a")
                STPb = pp_m.tile([1, H1], f32, tag="m", name="STP1b")
            else:
                inc, outc, UE, NCH, W1a, Wp = H1, H2, U2E, NC2, W1a2, Wp2
                uw = U2
                STPa = pp_m.tile([H2, 1], f32, tag="m", name="STP2a")
                STPb = pp_m.tile([H2, 1], f32, tag="m", name="STP2b")
            pre_list = []
            sq_list = []
            win_state = []
            # slot-rotated PSUM tiles (PSUM slots are bank-granular)
            UT = pp_u.tile([P, 8, P], f32, tag="ut")
            PRE = pp_pre.tile([P, 4, P], f32, tag="pre")
            for w in range(WPC):
                u_tiles = []
                oh_tiles = []
                for t3 in range(T):
                    t = w * T + t3
                    oh = ohall[:, t * P : (t + 1) * P]
                    if layer == 1:
                        u = u1_slab[:, t, :]
                    else:
                        z = z2_all[:, t, :]
                        # gather xs rows from the AllGathered h1 halves.
                        # Mixed tiles gather each half into a zeroed tile and
                        # add (OOB rows are skipped on hw / zeroed in sim —
                        # both give 0 for the missing half).
                        tt = ttypes[t]
                        if tt == 0:
                            xs = sb.tile([P, inc], bf16, tag="xs", bufs=12)
                            nc.gpsimd.indirect_dma_start(
                                out=xs[:], out_offset=None, in_=h1A[:],
                                in_offset=IndirectOffsetOnAxis(ap=srchA[:, t : t + 1], axis=0),
                            )
                        elif tt == 1:
                            xs = sb.tile([P, inc], bf16, tag="xs", bufs=12)
                            nc.gpsimd.indirect_dma_start(
                                out=xs[:], out_offset=None, in_=h1B[:],
                                in_offset=IndirectOffsetOnAxis(ap=srchB[:, t : t + 1], axis=0),
                            )
                        else:
                            xsA = sb.tile([P, inc], bf16, tag="xsA", bufs=4)
                            nc.gpsimd.memset(xsA[:], 0.0)
                            nc.gpsimd.indirect_dma_start(
                                out=xsA[:], out_offset=None, in_=h1A[:],
                                in_offset=IndirectOffsetOnAxis(ap=srchA[:, t : t + 1], axis=0),
                                bounds_check=NCORES * HA - 1,
                                oob_is_err=False,
                            )
                            xsB = sb.tile([P, inc], bf16, tag="xsB", bufs=4)
                            nc.gpsimd.memset(xsB[:], 0.0)
                            nc.gpsimd.indirect_dma_start(
                                out=xsB[:], out_offset=None, in_=h1B[:],
                                in_offset=IndirectOffsetOnAxis(ap=srchB[:, t : t + 1], axis=0),
                                bounds_check=NCORES * HB - 1,
                                oob_is_err=False,
                            )
                            xs = sb.tile([P, inc], bf16, tag="xs", bufs=12)
                            nc.vector.tensor_add(out=xs[:], in0=xsA[:], in1=xsB[:])
                        # xss = (xs*icnt)*bn_scale + bn_shift*icnt  (bf16 out)
                        xsi = sb.tile([P, inc], f32, tag="xsi", bufs=4)
                        nc.vector.scalar_tensor_tensor(
                            out=xsi[:], in0=xs[:], scalar=icnt[:, t : t + 1],
                            in1=scale_bc[:], op0=AL.mult, op1=AL.mult,
                        )
                        u = u2_slab[:, t % 8, :]
                        xss = u[:, uw : uw + inc]
                        nc.vector.scalar_tensor_tensor(
                            out=xss, in0=shift_bc[:], scalar=icnt[:, t : t + 1],
                            in1=xsi[:], op0=AL.mult, op1=AL.add,
                        )
                        # u = [z (x) xss | xss | zero-tail (slab preset)]
                        nc.vector.tensor_tensor(
                            out=u[:, :uw].rearrange("p (k i) -> p k i", k=KH),
                            in0=z.unsqueeze(2).to_broadcast([P, KH, inc]),
                            in1=xss.unsqueeze(1).to_broadcast([P, KH, inc]),
                            op=AL.mult,
                        )
                    u_tiles.append(u)
                    oh_tiles.append(oh)

                # ---- finishing work for a window (node+root+relu+stats src) ----
                def finish_window(wf, uts_src):
                    pre = PRE[:, wf % 4, :outc]
                    if uts_src is not None:  # layer-1: node matmuls lag a window
                        for cn in range(NCH):
                            nc.tensor.matmul(
                                out=pre[:], lhsT=uts_src[:, cn, :], rhs=Wp[:, cn, :outc],
                                start=(cn == 0), stop=False,
                            )
                    if layer == 1:
                        nc.tensor.matmul(
                            out=pre[:], lhsT=xTa[:, wf * P : (wf + 1) * P], rhs=root1a[:],
                            start=False, stop=True,
                        )
                    else:
                        nc.tensor.matmul(
                            out=pre[:], lhsT=h1T_list[wf][:], rhs=root2a[:],
                            start=False, stop=True,
                        )
                    # relu(pre * vmask) masks pad nodes (vmask in {0,1})
                    if layer == 1:
                        pre_sb = stash.tile([P, outc], bf16, tag="pre1")
                    else:
                        pre_sb = stash.tile([P, outc], f32, tag="pre2")
                    nc.scalar.activation(
                        out=pre_sb[:], in_=pre[:], func=AF.Relu,
                        scale=vmask[:, wf : wf + 1],
                    )
                    sq = stash.tile([P, outc], bf16 if layer == 1 else f32, tag="sq")
                    nc.scalar.activation(out=sq[:], in_=pre_sb[:], func=AF.Square)
                    sq_list.append(sq)
                    pre_list.append(pre_sb)
                    if layer == 1:
                        # pre-BN h1 rows go straight out for the AllGather
                        nc.sync.dma_start(
                            out=h1_slice[wf * P : (wf + 1) * P, :], in_=pre_sb[:]
                        )
                    else:
                        # pool the un-normalized output (BN2 applied post-AR)
                        h2p = sb.tile([P, H2], bf16, tag="h2p")
                        nc.scalar.activation(
                            out=h2p[:], in_=pre_sb[:], func=AF.Copy,
                            scale=igc[:, wf : wf + 1],
                        )
                        half = 0 if wf < WPC // 2 else 1
                        nc.tensor.matmul(
                            out=gTp[half][:], lhsT=h2p[:],
                            rhs=ohgall[:, wf * NG : (wf + 1) * NG],
                            start=(wf in (0, WPC // 2)),
                            stop=(wf in (WPC // 2 - 1, WPC - 1)),
                        )
                        if wf == WPC // 2 - 1:
                            finA_sb = sb.tile([P, NG], f32, tag="finA")
                            nc.vector.tensor_copy(out=finA_sb[:], in_=gTp[0][:])
                            nc.sync.dma_start(out=finA_loc[:], in_=finA_sb[:])
                            nc.gpsimd.collective_compute(
                                "AllReduce", mybir.AluOpType.add,
                                replica_groups=groups,
                                ins=[finA_loc.opt()], outs=[finA_g.opt()],
                            )

                if layer == 1:
                    # all 5 chunks scattered, ONE cast, previous window's node
                    # phase runs during this window's scatter
                    for c in range(NCH):
                        UTc = UT[:, c, :]
                        for t3 in range(T):
                            nc.tensor.matmul(
                                out=UTc[:],
                                lhsT=u_tiles[t3][:, c * P : (c + 1) * P],
                                rhs=oh_tiles[t3][:],
                                start=(t3 == 0),
                                stop=(t3 == T - 1),
                            )
                    UTs5 = sb.tile([P, NCH, P], bf16, tag="uts5", bufs=3)
                    nc.scalar.copy(out=UTs5[:], in_=UT[:, :NCH, :])
                    win_state.append((w, UTs5))
                    if len(win_state) > 1:
                        wf, uts_prev = win_state.pop(0)
                        finish_window(wf, uts_prev)
                else:
                    # quad casts, node matmuls lag 8 chunks behind the scatter
                    uts_quads = []
                    nodes_done = [0]

                    def emit_node(cn):
                        q, rq = divmod(cn, 4)
                        nc.tensor.matmul(
                            out=PRE[:, w % 4, :outc], lhsT=uts_quads[q][:, rq, :],
                            rhs=Wp[:, cn, :outc],
                            start=(cn == 0), stop=False,
                        )

                    for c in range(NCH):
                        UTc = UT[:, c % 8, :]
                        for t3 in range(T):
                            nc.tensor.matmul(
                                out=UTc[:],
                                lhsT=u_tiles[t3][:, c * P : (c + 1) * P],
                                rhs=oh_tiles[t3][:],
                                start=(t3 == 0),
                                stop=(t3 == T - 1),
                            )
                        if c % 4 == 3:
                            UTs = sb.tile([P, 4, P], bf16, tag="uts", bufs=4)
                            nc.scalar.copy(
                                out=UTs[:], in_=UT[:, (c - 3) % 8 : (c - 3) % 8 + 4, :]
                            )
                            uts_quads.append(UTs)
                        elif c == NCH - 1:
                            UTs = sb.tile([P, 4, P], bf16, tag="uts", bufs=4)
                            nc.scalar.copy(out=UTs[:, 0, :], in_=UTc[:])
                            uts_quads.append(UTs)
                        while nodes_done[0] <= c - 8:
                            emit_node(nodes_done[0])
                            nodes_done[0] += 1
                    while nodes_done[0] < NCH:
                        emit_node(nodes_done[0])
                        nodes_done[0] += 1
                    finish_window(w, None)
            while win_state:
                wf, uts_prev = win_state.pop(0)
                finish_window(wf, uts_prev)
            for w in range(WPC):
                if layer == 1:
                    nc.tensor.matmul(out=STPa[:], lhsT=onesPb[:], rhs=pre_list[w][:],
                                     start=(w == 0), stop=(w == WPC - 1))
                    nc.tensor.matmul(out=STPb[:], lhsT=onesPb[:], rhs=sq_list[w][:],
                                     start=(w == 0), stop=(w == WPC - 1))
                else:
                    nc.tensor.matmul(out=STPa[:], lhsT=pre_list[w][:], rhs=onesP[:],
                                     start=(w == 0), stop=(w == WPC - 1))
                    nc.tensor.matmul(out=STPb[:], lhsT=sq_list[w][:], rhs=onesP[:],
                                     start=(w == 0), stop=(w == WPC - 1))
            return pre_list, STPa, STPb

        # ======================= layer 1 =======================
        pre1, STP1a, STP1b = conv_layer(1)
        stats_sb1 = sb.tile([1, 2 * H1], f32, tag="st1")
        nc.vector.tensor_copy(out=stats_sb1[:, :H1], in_=STP1a[:])
        nc.vector.tensor_copy(out=stats_sb1[:, H1:], in_=STP1b[:])
        # split into bf16 hi/lo pairs (double-bf16) to ride the bf16 AllGather
        st_hi = sb.tile([1, 2 * H1], bf16, tag="sthi")
        nc.vector.tensor_copy(out=st_hi[:], in_=stats_sb1[:])
        st_lo = sb.tile([1, 2 * H1], bf16, tag="stlo")
        nc.vector.tensor_sub(out=st_lo[:], in0=stats_sb1[:], in1=st_hi[:])
        for r2 in range(2):
            nc.sync.dma_start(
                out=h1_slice[NPADC + r2 : NPADC + r2 + 1, :],
                in_=st_hi[:, r2 * H1 : (r2 + 1) * H1],
            )
            nc.sync.dma_start(
                out=h1_slice[NPADC + 2 + r2 : NPADC + 3 + r2, :],
                in_=st_lo[:, r2 * H1 : (r2 + 1) * H1],
            )
        nc.gpsimd.collective_compute(
            "AllGather", mybir.AluOpType.bypass, replica_groups=groups,
            ins=[h1_slice[:HA, :].opt()], outs=[h1A.opt()],
        )
        nc.gpsimd.collective_compute(
            "AllGather", mybir.AluOpType.bypass, replica_groups=groups,
            ins=[h1_slice[HA:, :].opt()], outs=[h1B.opt()],
        )

        # ---- overlapped with the AllGather: L2 z tiles + one-hots ----
        ZP2 = pp_z.tile([P, 4, KH], f32, tag="z")
        for t in range(NT):
            s0 = t * P
            zp2 = ZP2[:, t % 4, :]
            nc.tensor.matmul(
                out=zp2[:], lhsT=eaT[:, s0 : s0 + P], rhs=W1a2[:],
                start=True, stop=True,
            )
            nc.vector.tensor_scalar_max(out=z2_all[:, t, :], in0=zp2[:], scalar1=0.0)
        # ---- overlapped with the AllGather: h1^T transposes (pre-affine) ----
        h1T_raw = []
        TP = pp_u.tile([H1, 4, P], bf16, tag="ut")
        for w in range(WPC):
            tp = TP[:, w % 4, :]
            nc.tensor.transpose(out=tp[:], in_=pre1[w][:], identity=identb[:])
            tr = stash.tile([H1, P], f32, tag="h1Traw")
            nc.vector.tensor_copy(out=tr[:], in_=tp[:])
            h1T_raw.append(tr)

        # ---- reduce the 8 cores' stats rows (all on partition 0) ----
        sgat = sb.tile([1, NCORES, 4 * H1], bf16, tag="sgat")
        stat_rows = h1B[:].rearrange("(c r) f -> c r f", r=HB)[:, HB - 4 :, :]
        nc.sync.dma_start(
            out=sgat[:], in_=stat_rows.rearrange("c r f -> c (r f)").unsqueeze(0)
        )
        acc = sb.tile([1, 2 * H1], f32, tag="sacc")
        nc.vector.tensor_add(
            out=acc[:], in0=sgat[:, 0, : 2 * H1], in1=sgat[:, 0, 2 * H1 :]
        )
        for c in range(1, NCORES):
            nc.vector.tensor_add(out=acc[:], in0=acc[:], in1=sgat[:, c, : 2 * H1])
            nc.vector.tensor_add(out=acc[:], in0=acc[:], in1=sgat[:, c, 2 * H1 :])
        # bn1 coeffs (row orientation [1, H1])
        mu = sb.tile([1, H1], f32, tag="mu")
        nc.vector.tensor_scalar_mul(out=mu[:], in0=acc[:, :H1], scalar1=1.0 / N)
        va = sb.tile([1, H1], f32, tag="va")
        nc.vector.tensor_scalar_mul(out=va[:], in0=acc[:, H1:], scalar1=1.0 / N)
        musq = sb.tile([1, H1], f32, tag="musq")
        nc.vector.tensor_mul(out=musq[:], in0=mu[:], in1=mu[:])
        nc.vector.tensor_sub(out=va[:], in0=va[:], in1=musq[:])
        nc.vector.tensor_scalar_add(out=va[:], in0=va[:], scalar1=EPS)
        sd = sb.tile([1, H1], f32, tag="sd")
        nc.scalar.activation(out=sd[:], in_=va[:], func=AF.Sqrt)
        rs = sb.tile([1, H1], f32, tag="rs")
        nc.vector.reciprocal(out=rs[:], in_=sd[:])
        sc_r = sb.tile([1, H1], f32, tag="sc_r")
        nc.vector.tensor_mul(out=sc_r[:], in0=rs[:], in1=bng1r[:])
        sh_r = sb.tile([1, H1], f32, tag="sh_r")
        nc.vector.tensor_mul(out=sh_r[:], in0=mu[:], in1=sc_r[:])
        nc.vector.tensor_sub(out=sh_r[:], in0=bnb1r[:], in1=sh_r[:])
        # broadcast [P, H1] tiles for the folded gather affine
        scp = pp_m.tile([P, H1], f32, tag="m")
        nc.tensor.matmul(out=scp[:], lhsT=onesr[:], rhs=sc_r[:], start=True, stop=True)
        scale_bc = cst.tile([P, H1], f32, tag="scale_bc")
        nc.vector.tensor_copy(out=scale_bc[:], in_=scp[:])
        shp = pp_m.tile([P, H1], f32, tag="m")
        nc.tensor.matmul(out=shp[:], lhsT=onesr[:], rhs=sh_r[:], start=True, stop=True)
        shift_bc = cst.tile([P, H1], f32, tag="shift_bc")
        nc.vector.tensor_copy(out=shift_bc[:], in_=shp[:])
        # column coeffs [H1, 1] for the transposed h1 (root2 operand)
        sccp = pp_m.tile([H1, 1], f32, tag="m")
        nc.tensor.transpose(out=sccp[:], in_=sc_r[:], identity=ident[:1, :1])
        sc_c = sb.tile([H1, 1], f32, tag="sc_c")
        nc.vector.tensor_copy(out=sc_c[:], in_=sccp[:])
        shcp = pp_m.tile([H1, 1], f32, tag="m")
        nc.tensor.transpose(out=shcp[:], in_=sh_r[:], identity=ident[:1, :1])
        sh_c = sb.tile([H1, 1], f32, tag="sh_c")
        nc.vector.tensor_copy(out=sh_c[:], in_=shcp[:])
        # normalized h1^T per window with ones row (root2+bias2 operand)
        h1T_list = []
        for w in range(WPC):
            h1Ta = stash.tile([H1 + 1, P], bf16, tag="h1Ta")
            nc.vector.tensor_scalar(
                out=h1Ta[:H1, :], in0=h1T_raw[w][:], scalar1=sc_c[:, :1],
                scalar2=sh_c[:, :1], op0=AL.mult, op1=AL.add,
            )
            nc.gpsimd.memset(h1Ta[H1 : H1 + 1, :], 1.0)
            h1T_list.append(h1Ta)

        # ======================= layer 2 =======================
        gTp[0] = pp_g.tile([P, NG], f32, tag="gtp", name="gtpA")
        gTp.append(pp_g.tile([P, NG], f32, tag="gtpB", name="gtpB"))
        _, stp2a, stp2b = conv_layer(2, scale_bc, shift_bc, h1T_list)

        # one final AllReduce carries pooled graph features + BN2 stats
        fin_sb = sb.tile([P, NG + 2], f32, tag="fin")
        nc.vector.tensor_copy(out=fin_sb[:, :NG], in_=gTp[1][:])
        nc.vector.tensor_copy(out=fin_sb[:, NG : NG + 1], in_=stp2a[:])
        nc.vector.tensor_copy(out=fin_sb[:, NG + 1 : NG + 2], in_=stp2b[:])
        nc.sync.dma_start(out=fin_loc[:], in_=fin_sb[:])
        nc.gpsimd.collective_compute(
            "AllReduce", mybir.AluOpType.add, replica_groups=groups,
            ins=[fin_loc.opt()], outs=[fin_g.opt()],
        )
        fin = sb.tile([P, NG + 2], f32, tag="fin2")
        nc.sync.dma_start(out=fin[:], in_=fin_g[:])
        finA_rb = sb.tile([P, NG], f32, tag="finArb")
        nc.sync.dma_start(out=finA_rb[:], in_=finA_g[:])
        nc.vector.tensor_add(out=fin[:, :NG], in0=fin[:, :NG], in1=finA_rb[:])
        # bn2 coeffs (column orientation [H2, 1])
        mu2 = sb.tile([H2, 1], f32, tag="mu2")
        nc.vector.tensor_scalar_mul(out=mu2[:], in0=fin[:, NG : NG + 1], scalar1=1.0 / N)
        va2 = sb.tile([H2, 1], f32, tag="va2")
        nc.vector.tensor_scalar_mul(out=va2[:], in0=fin[:, NG + 1 : NG + 2], scalar1=1.0 / N)
        musq2 = sb.tile([H2, 1], f32, tag="musq2")
        nc.vector.tensor_mul(out=musq2[:], in0=mu2[:], in1=mu2[:])
        nc.vector.tensor_sub(out=va2[:], in0=va2[:], in1=musq2[:])
        nc.vector.tensor_scalar_add(out=va2[:], in0=va2[:], scalar1=EPS)
        sd2 = sb.tile([H2, 1], f32, tag="sd2")
        nc.scalar.activation(out=sd2[:], in_=va2[:], func=AF.Sqrt)
        rs2 = sb.tile([H2, 1], f32, tag="rs2")
        nc.vector.reciprocal(out=rs2[:], in_=sd2[:])
        sc2 = sb.tile([H2, 1], f32, tag="sc2")
        nc.vector.tensor_mul(out=sc2[:], in0=rs2[:], in1=bng2[:])
        sh2 = sb.tile([H2, 1], f32, tag="sh2")
        nc.vector.tensor_mul(out=sh2[:], in0=mu2[:], in1=sc2[:])
        nc.vector.tensor_sub(out=sh2[:], in0=bnb2[:], in1=sh2[:])
        # g = sc2 * g_raw + sh2 * gmask   (BN2 folded through the pool)
        gt = sb.tile([P, NG], f32, tag="gt")
        nc.vector.tensor_scalar_mul(out=gt[:], in0=fin[:, :NG], scalar1=sc2[:, :1])
        nc.vector.scalar_tensor_tensor(
            out=gt[:], in0=gmaskb[:], scalar=sh2[:, :1], in1=gt[:],
            op0=AL.mult, op1=AL.add,
        )

        # ======================= final MLP =======================
        l1p = pp_m.tile([H1, NG], f32, tag="m")
        nc.tensor.matmul(out=l1p[:], lhsT=l1W[:], rhs=gt[:], start=True, stop=True)
        hl = sb.tile([H1, NG], f32, tag="hl")
        nc.vector.tensor_scalar(
            out=hl[:], in0=l1p[:], scalar1=l1b[:, :1], scalar2=0.0,
            op0=AL.add, op1=AL.max,
        )
        l2p = pp_m.tile([1, NG], f32, tag="m")
        nc.tensor.matmul(out=l2p[:], lhsT=l2W[:], rhs=hl[:], start=True, stop=True)
        osb = sb.tile([1, NG], f32, tag="osb")
        nc.vector.tensor_scalar_add(out=osb[:], in0=l2p[:], scalar1=l2b[:, :1])
        nc.sync.dma_start(out=out_d[:], in_=osb[:])

    nc.compile()
    return nc


_CACHE = {}


def _get_program(T, ES, ttypes):
    key = (T, ES, ttypes)
    if key not in _CACHE:
        _CACHE[key] = build_program(T, ES, ttypes)
    return _CACHE[key]


def make_in_maps(inputs):
    pp = _preprocess(
        inputs["x"], inputs["edge_index"], inputs["edge_attr"], inputs["batch"]
    )
    w = _weights(inputs)
    bf = ml_dtypes.bfloat16
    shared = dict(
        W1a1=w["W1a1"].astype(bf), W1a2=w["W1a2"].astype(bf),
        Wp1=w["Wp1"].astype(bf), Wp2=w["Wp2"].astype(bf),
        root1a=w["root1a"].astype(bf), root2a=w["root2a"].astype(bf),
        bng1r=w["bng1r"], bnb1r=w["bnb1r"], bng2=w["bng2"], bnb2=w["bnb2"],
        l1W=w["l1W"], l1b=w["l1b"], l2W=w["l2W"], l2b=w["l2b"],
        onesP=w["onesP"], onesr=w["onesr"],
        gmaskb=np.ascontiguousarray(
            np.broadcast_to(pp["gmask"], (P, NG)).astype(np.float32)
        ),
    )
    in_maps = []
    for c in range(NCORES):
        m = dict(shared)
        m["eaT"] = np.ascontiguousarray(pp["eaT"][c].astype(bf))
        m["srchA"] = np.ascontiguousarray(pp["srchA"][c])
        m["srchB"] = np.ascontiguousarray(pp["srchB"][c])
        m["ohall"] = np.ascontiguousarray(pp["ohall"][c])
        m["ohgall"] = np.ascontiguousarray(pp["ohgall"][c])
        m["icnt"] = np.ascontiguousarray(pp["icnt"][c])
        m["xsrc"] = np.ascontiguousarray(pp["xsrc"][c])
        m["xTa"] = np.ascontiguousarray(pp["xTa"][c].astype(bf))
        m["igc"] = np.ascontiguousarray(pp["igc"][c])
        m["vmask"] = np.ascontiguousarray(pp["vmask"][c])
        in_maps.append(m)
    return in_maps, pp["T"], pp["ES"], pp["ttypes"]


def _run(inputs, trace=False):
    in_maps, T, ES, ttypes = make_in_maps(inputs)
    nc = _get_program(T, ES, ttypes)
    res = run_bass_kernel_spmd(
        nc, in_maps, core_ids=list(range(NCORES)), trace=trace
    )
    out = np.asarray(res.results[0]["out"][0], dtype=np.float32)
    return out, res


def kernel(**inputs):
    return _run(inputs)[0]


# revision 47
# speedup vs baseline: 1.0237x; 1.0237x over previous
"""MPNN (2x NNConv + BN + global mean pool + MLP) on 8 Trainium2 cores.

Strategy (node-sharded message passing), v2:
  * Never materialize We=[E,in_c,out_c].  msg[e] = (z[e] (x) xs[e]) @ W2r
    where z=relu(ea@W1+b1), xs=x[src], W2r = reshape of W2.  Since the
    segment-sum over dst commutes with the (shared) @W2r, we scatter the
    per-edge outer products u[e]=(z (x) xs_scaled) into per-node U first,
    then do ONE matmul per node tile:  agg = U @ W2r.
  * Nodes are bin-packed into 128-node windows balanced by edge count;
    each core owns 10 windows.  Edges are grouped per window (padded to
    T tiles of 128).  Scatter = one-hot matmul on the PE accumulating
    U^T chunks in PSUM, pipelined per 128-wide chunk with the
    PSUM->SBUF cast (scalar engine) and the node matmul.
  * v2 layout changes vs v1: all per-edge tables are preloaded once as
    resident SBUF tiles (no per-tile DMAs); x[src] for layer 1 is
    gathered host-side (pure indexing) so layer 1 has no indirect DMAs;
    bias1/bias2 ride as an extra ones-row on the root operand; BN1 is
    folded into the layer-2 gather; BN2+pool commute so BN2 is applied
    after the final AllReduce.  Collectives use Shared outputs; the
    layer-2 edge-MLP z tiles / one-hots / h1^T transposes are computed
    during the h1 AllGather.
"""

import sys

import numpy as np

try:
    import concourse.bass as bass  # noqa
except Exception:  # pragma: no cover
    sys.path.insert(0, "/opt/trn_rl_repo")

import ml_dtypes
import concourse.bacc as bacc
import concourse.bass as bass
import concourse.mybir as mybir
import concourse.tile as tile
from concourse.bass import IndirectOffsetOnAxis
from concourse.bass_utils import run_bass_kernel_spmd
from concourse.masks import make_identity

P = 128
NCORES = 8
N = 10000
E = 30000
NG = 256
IN_C = 16
EDGE_C = 8
KH = 32  # edge-MLP hidden width
H1 = 64  # conv1 out channels
H2 = 128  # conv2 out channels
WPC = 10  # windows per core
NPADC = WPC * P  # padded nodes per core (1280)
NSTR = NPADC + 4  # h1 slice rows per core incl. 4 BN-stats rows (f32 bits in bf16 pairs)
EPS = 1e-5
f32 = mybir.dt.float32
bf16 = mybir.dt.bfloat16
i32 = mybir.dt.int32

U1 = KH * IN_C  # 512
U1E = 640  # padded to 5 chunks of 128 (u | xs | zeros)
U2 = KH * H1  # 2048
U2E = 2176  # padded to 17 chunks of 128 (u | xs | zeros)


# --------------------------------------------------------------------------
# host-side preprocessing: index/layout work only
# --------------------------------------------------------------------------
def _preprocess(x, edge_index, edge_attr, batch):
    import heapq

    src = np.asarray(edge_index[0], dtype=np.int64)
    dst = np.asarray(edge_index[1], dtype=np.int64)
    deg = np.bincount(dst, minlength=N).astype(np.int64)

    # ---- bin-pack nodes into NCORES*WPC windows of exactly <=128 nodes,
    # balancing per-window edge counts (LPT greedy) ----
    NW = NCORES * WPC
    order = np.argsort(-deg, kind="stable")
    wsum = np.zeros(NW, dtype=np.int64)
    wcnt = np.zeros(NW, dtype=np.int64)
    win_of = np.empty(N, dtype=np.int64)
    slot_of = np.empty(N, dtype=np.int64)
    heap = [(0, w) for w in range(NW)]
    heapq.heapify(heap)
    for n in order:
        while True:
            _, w = heapq.heappop(heap)
            if wcnt[w] < P:
                break
        win_of[n] = w
        slot_of[n] = wcnt[w]
        wcnt[w] += 1
        wsum[w] += deg[n]
        if wcnt[w] < P:
            heapq.heappush(heap, (int(wsum[w]), w))

    T = max(1, int(-(-int(wsum.max()) // P)))  # tiles (of 128 edges) per window
    NT = WPC * T  # edge tiles per core
    ES = NT * P  # edge slots per core

    core_of = win_of // WPC
    lpos = (win_of % WPC) * P + slot_of

    # ---- per-edge placement ----
    ew = win_of[dst]
    eorder = np.argsort(ew, kind="stable")
    inv_cnt = 1.0 / np.maximum(deg, 1).astype(np.float32)

    ea_s = np.zeros((NCORES, ES, EDGE_C), dtype=np.float32)
    ones_s = np.zeros((NCORES, ES), dtype=np.float32)
    srcx_s = np.zeros((NCORES, ES), dtype=np.int64)
    srch_s = np.zeros((NCORES, ES), dtype=np.int32)
    dstrel_s = np.full((NCORES, ES), -1.0, dtype=np.float32)
    icnt_s = np.zeros((NCORES, ES), dtype=np.float32)

    ew_sorted = ew[eorder]
    starts = np.searchsorted(ew_sorted, np.arange(NW))
    ends = np.searchsorted(ew_sorted, np.arange(NW) + 1)
    HA = 5 * P  # rows in the first AllGather half
    HB = NSTR - HA
    SENT = 1 << 20
    srchA_s = np.zeros((NCORES, ES), dtype=np.int32)
    srchB_s = np.zeros((NCORES, ES), dtype=np.int32)
    isA_s = np.zeros((NCORES, ES), dtype=bool)
    isB_s = np.zeros((NCORES, ES), dtype=bool)
    ea_np = np.asarray(edge_attr, dtype=np.float32)
    for w in range(NW):
        es = eorder[starts[w] : ends[w]]
        # sort the window's edges: sources in half A first, then half B
        eA = lpos[src[es]] < HA
        es = es[np.argsort(~eA, kind="stable")]
        c = w // WPC
        base = (w % WPC) * T * P
        k = len(es)
        assert k <= T * P
        sl = slice(base, base + k)
        ea_s[c, sl] = ea_np[es]
        ones_s[c, sl] = 1.0
        srcx_s[c, sl] = src[es]
        _lp = lpos[src[es]]
        _co = core_of[src[es]]
        _inA = _lp < HA
        srch_s[c, sl] = np.where(
            _inA, _co * HA + _lp, NCORES * HA + _co * HB + (_lp - HA)
        ).astype(np.int32)
        srchA_s[c, sl] = np.where(_inA, _co * HA + _lp, SENT).astype(np.int32)
        srchB_s[c, sl] = np.where(~_inA, _co * HB + (_lp - HA), SENT).astype(np.int32)
        isA_s[c, sl] = _inA
        isB_s[c, sl] = ~_inA
        dstrel_s[c, sl] = slot_of[dst[es]]
        icnt_s[c, sl] = inv_cnt[dst[es]]
    # pad slots: valid row 0 in both tables (values are zeroed by icnt=0)
    pad = ~(isA_s | isB_s)
    srchA_s[pad] = 0
    srchB_s[pad] = 0
    # per-tile gather type, unioned across cores (the program is SPMD):
    # 0 = pure-A, 1 = pure-B, 2 = mixed
    hasA = isA_s.reshape(NCORES, NT, P).any(axis=(0, 2))
    hasB = isB_s.reshape(NCORES, NT, P).any(axis=(0, 2))
    ttypes = tuple(
        (2 if (a and b) else (1 if b else 0)) for a, b in zip(hasA, hasB)
    )

    eaT = np.concatenate(
        [np.transpose(ea_s, (0, 2, 1)), ones_s[:, None, :]], axis=1
    ).astype(np.float32)

    # host gather of x[src] (pure indexing), tile-major [P, NT*IN_C]
    x_np = np.asarray(x, dtype=np.float32)
    xsrc = x_np[srcx_s.reshape(NCORES, NT, P)]  # [C, NT, P, IN_C]
    xsrc_s = np.ascontiguousarray(
        xsrc.transpose(0, 2, 1, 3).reshape(NCORES, P, NT * IN_C)
    )

    # per-edge tables in [P(slot-in-tile), NT] layout
    def t_major(a):
        return np.ascontiguousarray(
            a.reshape(NCORES, NT, P).transpose(0, 2, 1)
        )

    srch_t = t_major(srch_s)
    srchA_t = t_major(srchA_s)
    srchB_t = t_major(srchB_s)
    drel_t = t_major(dstrel_s)
    icnt_t = t_major(icnt_s)
    ohall = (drel_t[:, :, :, None] == np.arange(P, dtype=np.float32)).astype(
        ml_dtypes.bfloat16
    ).reshape(NCORES, P, NT * P)

    # ---- per-node per-core tables ----
    batch = np.asarray(batch, dtype=np.int64)
    gcnt = np.bincount(batch, minlength=NG).astype(np.int64)
    igc_node = (1.0 / np.maximum(gcnt, 1).astype(np.float32))[batch]
    gmask = (gcnt > 0).astype(np.float32)

    xTa_s = np.zeros((NCORES, IN_C + 1, NPADC), dtype=np.float32)
    xTa_s[:, IN_C, :] = 1.0  # ones row for fused bias
    batch_s = np.full((NCORES, P, WPC), -1.0, dtype=np.float32)
    igc_s = np.zeros((NCORES, P, WPC), dtype=np.float32)
    vmask_s = np.zeros((NCORES, P, WPC), dtype=np.float32)
    for c in range(NCORES):
        m = core_of == c
        lp = lpos[m]
        xTa_s[c][:IN_C, lp] = x_np[m].T
        batch_s[c][lp % P, lp // P] = batch[m].astype(np.float32)
        igc_s[c][lp % P, lp // P] = igc_node[m]
        vmask_s[c][lp % P, lp // P] = 1.0

    ohgall = (batch_s[:, :, :, None] == np.arange(NG, dtype=np.float32)).astype(
        ml_dtypes.bfloat16
    ).reshape(NCORES, P, WPC * NG)

    return dict(
        T=T, ES=ES, NT=NT, eaT=eaT, srch=srch_t, srchA=srchA_t, srchB=srchB_t,
        ttypes=ttypes, drel=drel_t, icnt=icnt_t,
        xsrc=xsrc_s, xTa=xTa_s, batchrel=batch_s, igc=igc_s, vmask=vmask_s,
        gmask=gmask, ohall=ohall, ohgall=ohgall,
    )


def _weights(p):
    w = {}
    w["W1a1"] = np.concatenate([p["nn1_W1"], p["nn1_b1"][None, :]], 0).astype(np.float32)
    w["W1a2"] = np.concatenate([p["nn2_W1"], p["nn2_b1"][None, :]], 0).astype(np.float32)
    wp1 = np.zeros((U1E, H1), dtype=np.float32)
    wp1[:U1] = p["nn1_W2"].reshape(KH, IN_C, H1).reshape(U1, H1)
    wp1[U1 : U1 + IN_C] = p["nn1_b2"].reshape(IN_C, H1)
    w["Wp1"] = wp1
    wp2 = np.zeros((U2E, H2), dtype=np.float32)
    wp2[:U2] = p["nn2_W2"].reshape(KH, H1, H2).reshape(U2, H2)
    wp2[U2 : U2 + H1] = p["nn2_b2"].reshape(H1, H2)
    w["Wp2"] = wp2
    w["root1a"] = np.concatenate(
        [np.asarray(p["root1"], np.float32), np.asarray(p["bias1"], np.float32)[None, :]], 0
    )
    w["root2a"] = np.concatenate(
        [np.asarray(p["root2"], np.float32), np.asarray(p["bias2"], np.float32)[None, :]], 0
    )
    w["bng1r"] = np.asarray(p["bn1_g"], np.float32)[None, :]
    w["bnb1r"] = np.asarray(p["bn1_b"], np.float32)[None, :]
    w["bng2"] = np.asarray(p["bn2_g"], np.float32)[:, None]
    w["bnb2"] = np.asarray(p["bn2_b"], np.float32)[:, None]
    w["l1W"] = np.asarray(p["lin1_W"], np.float32)
    w["l1b"] = np.asarray(p["lin1_b"], np.float32)[:, None]
    w["l2W"] = np.asarray(p["lin2_W"], np.float32)
    w["l2b"] = np.asarray(p["lin2_b"], np.float32)[None, :]
    w["iota128"] = np.broadcast_to(np.arange(P, dtype=np.float32), (P, P)).copy()
    w["iota256"] = np.broadcast_to(np.arange(NG, dtype=np.float32), (P, NG)).copy()
    w["onesP"] = np.ones((P, 1), dtype=np.float32)
    w["onesr"] = np.ones((1, P), dtype=np.float32)
    return w


# --------------------------------------------------------------------------
# device program (identical for all cores; per-core data comes via inputs)
# --------------------------------------------------------------------------
def build_program(T, ES, ttypes):
    NT = WPC * T
    HA = 5 * P
    HB = NSTR - HA
    AL = mybir.AluOpType
    AF = mybir.ActivationFunctionType
    nc = bacc.Bacc("TRN2", target_bir_lowering=False, debug=False, num_devices=NCORES)

    def din(name, shape, dtype=f32):
        return nc.dram_tensor(name, shape, dtype, kind="ExternalInput").ap()

    eaT_d = din("eaT", [EDGE_C + 1, ES], bf16)
    srchA_d = din("srchA", [P, NT], i32)
    srchB_d = din("srchB", [P, NT], i32)
    ohall_d = din("ohall", [P, NT * P], bf16)
    ohgall_d = din("ohgall", [P, WPC * NG], bf16)
    icnt_d = din("icnt", [P, NT])
    xsrc_d = din("xsrc", [P, NT * IN_C])
    xTa_d = din("xTa", [IN_C + 1, NPADC], bf16)
    igc_d = din("igc", [P, WPC])
    vmask_d = din("vmask", [P, WPC])
    W1a1_d = din("W1a1", [EDGE_C + 1, KH], bf16)
    W1a2_d = din("W1a2", [EDGE_C + 1, KH], bf16)
    Wp1_d = din("Wp1", [U1E, H1], bf16)
    Wp2_d = din("Wp2", [U2E, H2], bf16)
    root1a_d = din("root1a", [IN_C + 1, H1], bf16)
    root2a_d = din("root2a", [H1 + 1, H2], bf16)
    bng1r_d = din("bng1r", [1, H1])
    bnb1r_d = din("bnb1r", [1, H1])
    bng2_d = din("bng2", [H2, 1])
    bnb2_d = din("bnb2", [H2, 1])
    l1W_d = din("l1W", [H2, H1])
    l1b_d = din("l1b", [H1, 1])
    l2W_d = din("l2W", [H1, 1])
    l2b_d = din("l2b", [1, 1])
    onesP_d = din("onesP", [P, 1])
    onesr_d = din("onesr", [1, P])
    gmaskb_d = din("gmaskb", [P, NG])
    out_d = nc.dram_tensor("out", [1, NG], f32, kind="ExternalOutput").ap()

    NC1 = U1E // P  # 5 chunks
    NC2 = U2E // P  # 17 chunks

    from contextlib import ExitStack

    with tile.TileContext(nc) as tc, ExitStack() as pools:
        cst = pools.enter_context(tc.tile_pool(name="cst", bufs=1))
        sb = pools.enter_context(tc.tile_pool(name="sb", bufs=3))
        stash = pools.enter_context(tc.tile_pool(name="stash", bufs=WPC))
        pp_u = pools.enter_context(tc.tile_pool(name="pp_u", bufs=1, space="PSUM"))
        pp_pre = pools.enter_context(tc.tile_pool(name="pp_pre", bufs=1, space="PSUM"))
        pp_z = pools.enter_context(tc.tile_pool(name="pp_z", bufs=1, space="PSUM"))
        pp_m = pools.enter_context(tc.tile_pool(name="pp_m", bufs=2, space="PSUM"))
        pp_g = pools.enter_context(tc.tile_pool(name="pp_g", bufs=1, space="PSUM"))
        dram = pools.enter_context(tc.tile_pool(name="dram", bufs=1, space="DRAM"))

        # ---- resident constants (spread initial DMAs across engine queues) ----
        _eng = [nc.sync, nc.gpsimd]
        _ei = [0]

        def load(shape, ap, name, dt=f32):
            t = cst.tile(shape, dt, tag=name, name=name)
            _eng[_ei[0] % len(_eng)].dma_start(out=t[:], in_=ap)
            _ei[0] += 1
            return t

        groups = [list(range(NCORES))]
        warm_in = dram.tile([1, 1], f32, tag="warmi")
        warm_out = dram.tile([NCORES, 1], f32, tag="warmo", addr_space="Shared")
        nc.sync.dma_start(out=warm_in[:], in_=onesP_d[:1, :])
        nc.gpsimd.collective_compute(
            "AllGather", mybir.AluOpType.bypass, replica_groups=groups,
            ins=[warm_in.opt()], outs=[warm_out.opt()],
        )
        ident = cst.tile([P, P], f32, tag="ident")
        make_identity(nc, ident[:])
        identb = cst.tile([P, P], bf16, tag="identb")
        make_identity(nc, identb[:])
        # L1-critical tables first so windows can start ASAP
        eaT = load([EDGE_C + 1, ES], eaT_d[:], "eaT", bf16)
        W1a1 = load([EDGE_C + 1, KH], W1a1_d[:], "W1a1", bf16)
        icnt = load([P, NT], icnt_d[:], "icnt")
        xsrc = load([P, NT * IN_C], xsrc_d[:], "xsrc")
        Wp1 = cst.tile([P, NC1, H1], bf16, tag="Wp1")
        nc.gpsimd.dma_start(out=Wp1[:], in_=Wp1_d.rearrange("(c p) o -> p c o", p=P))
        ohall = load([P, NT * P], ohall_d[:], "ohall", bf16)
        xTa = load([IN_C + 1, NPADC], xTa_d[:], "xTa", bf16)
        root1a = load([IN_C + 1, H1], root1a_d[:], "root1a", bf16)
        vmask = load([P, WPC], vmask_d[:], "vmask")
        onesP = load([P, 1], onesP_d[:], "onesP")
        # L2 / tail tables
        W1a2 = load([EDGE_C + 1, KH], W1a2_d[:], "W1a2", bf16)
        Wp2 = cst.tile([P, NC2, H2], bf16, tag="Wp2")
        nc.scalar.dma_start(out=Wp2[:], in_=Wp2_d.rearrange("(c p) o -> p c o", p=P))
        srchA = load([P, NT], srchA_d[:], "srchA", i32)
        srchB = load([P, NT], srchB_d[:], "srchB", i32)
        ohgall = load([P, WPC * NG], ohgall_d[:], "ohgall", bf16)
        root2a = load([H1 + 1, H2], root2a_d[:], "root2a", bf16)
        bng1r = load([1, H1], bng1r_d[:], "bng1r")
        bnb1r = load([1, H1], bnb1r_d[:], "bnb1r")
        bng2 = load([H2, 1], bng2_d[:], "bng2")
        bnb2 = load([H2, 1], bnb2_d[:], "bnb2")
        l1W = load([H2, H1], l1W_d[:], "l1W")
        l1b = load([H1, 1], l1b_d[:], "l1b")
        l2W = load([H1, 1], l2W_d[:], "l2W")
        l2b = load([1, 1], l2b_d[:], "l2b")
        onesPb = cst.tile([P, 1], bf16, tag="onesPb")
        nc.vector.tensor_copy(out=onesPb[:], in_=onesP[:])
        onesr = load([1, P], onesr_d[:], "onesr")
        gmaskb = load([P, NG], gmaskb_d[:], "gmaskb")
        igc = load([P, WPC], igc_d[:], "igc")


        h1_slice = dram.tile([NSTR, H1], bf16, tag="h1s")
        h1A = dram.tile([NCORES * HA, H1], bf16, tag="h1a", addr_space="Shared")
        h1B = dram.tile([NCORES * HB, H1], bf16, tag="h1b", addr_space="Shared")
        finA_loc = dram.tile([P, NG], f32, tag="final")
        finA_g = dram.tile([P, NG], f32, tag="finag", addr_space="Shared")
        fin_loc = dram.tile([P, NG + 2], f32, tag="finl")
        fin_g = dram.tile([P, NG + 2], f32, tag="fing", addr_space="Shared")

        gTp = [None]

        # stash for precomputed per-tile z (bf16)
        z2_all = cst.tile([P, NT, KH], bf16, tag="z2all")

        # u slabs: layer 1 keeps ALL tiles resident (full prepass); layer 2
        # rotates 6 slots.  Zero tails initialized ONCE.
        u1_slab = cst.tile([P, NT, U1E], bf16, tag="u1slab")
        u2_slab = cst.tile([P, 6, U2E], bf16, tag="u2slab")
        for si in range(NT):
            nc.gpsimd.memset(u1_slab[:, si, U1 + IN_C :], 0.0)
        for si in range(6):
            nc.gpsimd.memset(u2_slab[:, si, U2 + H1 :], 0.0)

        # ---- L1 prepass: z, xss and outer products for ALL tiles ----
        ZP1 = pp_z.tile([P, 4, KH], f32, tag="z")
        for t in range(NT):
            s0 = t * P
            zp = ZP1[:, t % 4, :]
            nc.tensor.matmul(
                out=zp[:], lhsT=eaT[:, s0 : s0 + P], rhs=W1a1[:],
                start=True, stop=True,
            )
            z = sb.tile([P, KH], bf16, tag="z_sb", bufs=4)
            nc.vector.tensor_scalar_max(out=z[:], in0=zp[:], scalar1=0.0)
            u = u1_slab[:, t, :]
            xss = u[:, U1 : U1 + IN_C]
            nc.vector.tensor_scalar_mul(
                out=xss, in0=xsrc[:, t * IN_C : (t + 1) * IN_C],
                scalar1=icnt[:, t : t + 1],
            )
            nc.vector.tensor_tensor(
                out=u[:, :U1].rearrange("p (k i) -> p k i", k=KH),
                in0=z.unsqueeze(2).to_broadcast([P, KH, IN_C]),
                in1=xss.unsqueeze(1).to_broadcast([P, KH, IN_C]),
                op=AL.mult,
            )

        # ================= generic conv layer =================
        def conv_layer(layer, scale_bc=None, shift_bc=None, h1T_list=None):
            if layer == 1:
                inc, outc, UE, NCH, W1a, Wp = IN_C, H1, U1E, NC1, W1a1, Wp1
                uw = U1
                STPa = pp_m.tile([1, H1], f32, tag="m", name="STP1a")
                STPb = pp_m.tile([1, H1], f32, tag="m", name="STP1b")
            else:
                inc, outc, UE, NCH, W1a, Wp = H1, H2, U2E, NC2, W1a2, Wp2
                uw = U2
                STPa = pp_m.tile([H2, 1], f32, tag="m", name="STP2a")
                STPb = pp_m.tile([H2, 1], f32, tag="m", name="STP2b")
            pre_list = []
            sq_list = []
            win_state = []
            # slot-rotated PSUM tiles (PSUM slots are bank-granular)
            UT = pp_u.tile([P, 8, P], f32, tag="ut")
            PRE = pp_pre.tile([P, 4, P], f32, tag="pre")
            for w in range(WPC):
                u_tiles = []
                oh_tiles = []
                for t3 in range(T):
                    t = w * T + t3
                    oh = ohall[:, t * P : (t + 1) * P]
                    if layer == 1:
                        u = u1_slab[:, t, :]
                    else:
                        z = z2_all[:, t, :]
                        # gather xs rows from the AllGathered h1 halves.
                        # Mixed tiles gather each half into a zeroed tile and
                        # add (OOB rows are skipped on hw / zeroed in sim —
                        # both give 0 for the missing half).
                        tt = ttypes[t]
                        if tt == 0:
                            xs = sb.tile([P, inc], bf16, tag="xs", bufs=8)
                            nc.gpsimd.indirect_dma_start(
                                out=xs[:], out_offset=None, in_=h1A[:],
                                in_offset=IndirectOffsetOnAxis(ap=srchA[:, t : t + 1], axis=0),
                            )
                        elif tt == 1:
                            xs = sb.tile([P, inc], bf16, tag="xs", bufs=8)
                            nc.gpsimd.indirect_dma_start(
                                out=xs[:], out_offset=None, in_=h1B[:],
                                in_offset=IndirectOffsetOnAxis(ap=srchB[:, t : t + 1], axis=0),
                            )
                        else:
                            xsA = sb.tile([P, inc], bf16, tag="xsA", bufs=4)
                            nc.gpsimd.memset(xsA[:], 0.0)
                            nc.gpsimd.indirect_dma_start(
                                out=xsA[:], out_offset=None, in_=h1A[:],
                                in_offset=IndirectOffsetOnAxis(ap=srchA[:, t : t + 1], axis=0),
                                bounds_check=NCORES * HA - 1,
                                oob_is_err=False,
                            )
                            xsB = sb.tile([P, inc], bf16, tag="xsB", bufs=4)
                            nc.gpsimd.memset(xsB[:], 0.0)
                            nc.gpsimd.indirect_dma_start(
                                out=xsB[:], out_offset=None, in_=h1B[:],
                                in_offset=IndirectOffsetOnAxis(ap=srchB[:, t : t + 1], axis=0),
                                bounds_check=NCORES * HB - 1,
                                oob_is_err=False,
                            )
                            xs = sb.tile([P, inc], bf16, tag="xs", bufs=8)
                            nc.vector.tensor_add(out=xs[:], in0=xsA[:], in1=xsB[:])
                        # xss = (xs*icnt)*bn_scale + bn_shift*icnt  (bf16 out)
                        xsi = sb.tile([P, inc], f32, tag="xsi", bufs=4)
                        nc.vector.scalar_tensor_tensor(
                            out=xsi[:], in0=xs[:], scalar=icnt[:, t : t + 1],
                            in1=scale_bc[:], op0=AL.mult, op1=AL.mult,
                        )
                        u = u2_slab[:, t % 6, :]
                        xss = u[:, uw : uw + inc]
                        nc.vector.scalar_tensor_tensor(
                            out=xss, in0=shift_bc[:], scalar=icnt[:, t : t + 1],
                            in1=xsi[:], op0=AL.mult, op1=AL.add,
                        )
                        # u = [z (x) xss | xss | zero-tail (slab preset)]
                        nc.vector.tensor_tensor(
                            out=u[:, :uw].rearrange("p (k i) -> p k i", k=KH),
                            in0=z.unsqueeze(2).to_broadcast([P, KH, inc]),
                            in1=xss.unsqueeze(1).to_broadcast([P, KH, inc]),
                            op=AL.mult,
                        )
                    u_tiles.append(u)
                    oh_tiles.append(oh)

                # ---- finishing work for a window (node+root+relu+stats src) ----
                def finish_window(wf, uts_src):
                    pre = PRE[:, wf % 4, :outc]
                    if uts_src is not None:  # layer-1: node matmuls lag a window
                        for cn in range(NCH):
                            nc.tensor.matmul(
                                out=pre[:], lhsT=uts_src[:, cn, :], rhs=Wp[:, cn, :outc],
                                start=(cn == 0), stop=False,
                            )
                    if layer == 1:
                        nc.tensor.matmul(
                            out=pre[:], lhsT=xTa[:, wf * P : (wf + 1) * P], rhs=root1a[:],
                            start=False, stop=True,
                        )
                    else:
                        nc.tensor.matmul(
                            out=pre[:], lhsT=h1T_list[wf][:], rhs=root2a[:],
                            start=False, stop=True,
                        )
                    # relu(pre * vmask) masks pad nodes (vmask in {0,1})
                    if layer == 1:
                        pre_sb = stash.tile([P, outc], bf16, tag="pre1")
                    else:
                        pre_sb = stash.tile([P, outc], f32, tag="pre2")
                    nc.scalar.activation(
                        out=pre_sb[:], in_=pre[:], func=AF.Relu,
                        scale=vmask[:, wf : wf + 1],
                    )
                    sq = stash.tile([P, outc], bf16 if layer == 1 else f32, tag="sq")
                    nc.scalar.activation(out=sq[:], in_=pre_sb[:], func=AF.Square)
                    sq_list.append(sq)
                    pre_list.append(pre_sb)
                    if layer == 1:
                        # pre-BN h1 rows go straight out for the AllGather
                        nc.sync.dma_start(
                            out=h1_slice[wf * P : (wf + 1) * P, :], in_=pre_sb[:]
                        )
                    else:
                        # pool the un-normalized output (BN2 applied post-AR)
                        h2p = sb.tile([P, H2], bf16, tag="h2p")
                        nc.scalar.activation(
                            out=h2p[:], in_=pre_sb[:], func=AF.Copy,
                            scale=igc[:, wf : wf + 1],
                        )
                        half = 0 if wf < WPC // 2 else 1
                        nc.tensor.matmul(
                            out=gTp[half][:], lhsT=h2p[:],
                            rhs=ohgall[:, wf * NG : (wf + 1) * NG],
                            start=(wf in (0, WPC // 2)),
                            stop=(wf in (WPC // 2 - 1, WPC - 1)),
                        )
                        if wf == WPC // 2 - 1:
                            finA_sb = sb.tile([P, NG], f32, tag="finA")
                            nc.vector.tensor_copy(out=finA_sb[:], in_=gTp[0][:])
                            nc.sync.dma_start(out=finA_loc[:], in_=finA_sb[:])
                            nc.gpsimd.collective_compute(
                                "AllReduce", mybir.AluOpType.add,
                                replica_groups=groups,
                                ins=[finA_loc.opt()], outs=[finA_g.opt()],
                            )

                if layer == 1:
                    # all 5 chunks scattered, ONE cast, previous window's node
                    # phase runs during this window's scatter
                    for c in range(NCH):
                        UTc = UT[:, c, :]
                        for t3 in range(T):
                            nc.tensor.matmul(
                                out=UTc[:],
                                lhsT=u_tiles[t3][:, c * P : (c + 1) * P],
                                rhs=oh_tiles[t3][:],
                                start=(t3 == 0),
                                stop=(t3 == T - 1),
                            )
                    UTs5 = sb.tile([P, NCH, P], bf16, tag="uts5", bufs=3)
                    nc.scalar.copy(out=UTs5[:], in_=UT[:, :NCH, :])
                    win_state.append((w, UTs5))
                    if len(win_state) > 1:
                        wf, uts_prev = win_state.pop(0)
                        finish_window(wf, uts_prev)
                else:
                    # quad casts; node matmuls lag 8 chunks; the previous
                    # window's drain/root/relu/pool runs under this scatter
                    uts_quads = []
                    nodes_done = [0]

                    def emit_node2(wn, quads, cn):
                        q, rq = divmod(cn, 4)
                        nc.tensor.matmul(
                            out=PRE[:, wn % 4, :outc], lhsT=quads[q][:, rq, :],
                            rhs=Wp[:, cn, :outc],
                            start=(cn == 0), stop=False,
                        )

                    def drain_prev():
                        if win_state:
                            pw, p_quads, p_done = win_state.pop(0)
                            while p_done < NCH:
                                emit_node2(pw, p_quads, p_done)
                                p_done += 1
                            finish_window(pw, None)

                    for c in range(NCH):
                        UTc = UT[:, c % 8, :]
                        for t3 in range(T):
                            nc.tensor.matmul(
                                out=UTc[:],
                                lhsT=u_tiles[t3][:, c * P : (c + 1) * P],
                                rhs=oh_tiles[t3][:],
                                start=(t3 == 0),
                                stop=(t3 == T - 1),
                            )
                        if c % 4 == 3:
                            UTs = sb.tile([P, 4, P], bf16, tag="uts", bufs=5)
                            nc.scalar.copy(
                                out=UTs[:], in_=UT[:, (c - 3) % 8 : (c - 3) % 8 + 4, :]
                            )
                            uts_quads.append(UTs)
                        elif c == NCH - 1:
                            UTs = sb.tile([P, 4, P], bf16, tag="uts", bufs=5)
                            nc.scalar.copy(out=UTs[:, 0, :], in_=UTc[:])
                            uts_quads.append(UTs)
                        if c == 1:
                            drain_prev()
                        while nodes_done[0] <= c - 8:
                            emit_node2(w, uts_quads, nodes_done[0])
                            nodes_done[0] += 1
                    win_state.append((w, uts_quads, nodes_done[0]))
            while win_state:
                entry = win_state.pop(0)
                if layer == 1:
                    wf, uts_prev = entry
                    finish_window(wf, uts_prev)
                else:
                    pw, p_quads, p_done = entry
                    while p_done < NCH:
                        emit_node2(pw, p_quads, p_done)
                        p_done += 1
                    finish_window(pw, None)
            for w in range(WPC):
                if layer == 1:
                    nc.tensor.matmul(out=STPa[:], lhsT=onesPb[:], rhs=pre_list[w][:],
                                     start=(w == 0), stop=(w == WPC - 1))
                    nc.tensor.matmul(out=STPb[:], lhsT=onesPb[:], rhs=sq_list[w][:],
                                     start=(w == 0), stop=(w == WPC - 1))
                else:
                    nc.tensor.matmul(out=STPa[:], lhsT=pre_list[w][:], rhs=onesP[:],
                                     start=(w == 0), stop=(w == WPC - 1))
                    nc.tensor.matmul(out=STPb[:], lhsT=sq_list[w][:], rhs=onesP[:],
                                     start=(w == 0), stop=(w == WPC - 1))
            return pre_list, STPa, STPb

        # ======================= layer 1 =======================
        pre1, STP1a, STP1b = conv_layer(1)
        stats_sb1 = sb.tile([1, 2 * H1], f32, tag="st1")
        nc.vector.tensor_copy(out=stats_sb1[:, :H1], in_=STP1a[:])
        nc.vector.tensor_copy(out=stats_sb1[:, H1:], in_=STP1b[:])
        # split into bf16 hi/lo pairs (double-bf16) to ride the bf16 AllGather
        st_hi = sb.tile([1, 2 * H1], bf16, tag="sthi")
        nc.vector.tensor_copy(out=st_hi[:], in_=stats_sb1[:])
        st_lo = sb.tile([1, 2 * H1], bf16, tag="stlo")
        nc.vector.tensor_sub(out=st_lo[:], in0=stats_sb1[:], in1=st_hi[:])
        for r2 in range(2):
            nc.sync.dma_start(
                out=h1_slice[NPADC + r2 : NPADC + r2 + 1, :],
                in_=st_hi[:, r2 * H1 : (r2 + 1) * H1],
            )
            nc.sync.dma_start(
                out=h1_slice[NPADC + 2 + r2 : NPADC + 3 + r2, :],
                in_=st_lo[:, r2 * H1 : (r2 + 1) * H1],
            )
        nc.gpsimd.collective_compute(
            "AllGather", mybir.AluOpType.bypass, replica_groups=groups,
            ins=[h1_slice[:HA, :].opt()], outs=[h1A.opt()],
        )
        nc.gpsimd.collective_compute(
            "AllGather", mybir.AluOpType.bypass, replica_groups=groups,
            ins=[h1_slice[HA:, :].opt()], outs=[h1B.opt()],
        )

        # ---- overlapped with the AllGather: L2 z tiles + one-hots ----
        ZP2 = pp_z.tile([P, 4, KH], f32, tag="z")
        for t in range(NT):
            s0 = t * P
            zp2 = ZP2[:, t % 4, :]
            nc.tensor.matmul(
                out=zp2[:], lhsT=eaT[:, s0 : s0 + P], rhs=W1a2[:],
                start=True, stop=True,
            )
            nc.vector.tensor_scalar_max(out=z2_all[:, t, :], in0=zp2[:], scalar1=0.0)
        # ---- overlapped with the AllGather: h1^T transposes (pre-affine) ----
        h1T_raw = []
        TP = pp_u.tile([H1, 4, P], bf16, tag="ut")
        for w in range(WPC):
            tp = TP[:, w % 4, :]
            nc.tensor.transpose(out=tp[:], in_=pre1[w][:], identity=identb[:])
            tr = stash.tile([H1, P], f32, tag="h1Traw")
            nc.vector.tensor_copy(out=tr[:], in_=tp[:])
            h1T_raw.append(tr)

        # ---- reduce the 8 cores' stats rows (all on partition 0) ----
        sgat = sb.tile([1, NCORES, 4 * H1], bf16, tag="sgat")
        stat_rows = h1B[:].rearrange("(c r) f -> c r f", r=HB)[:, HB - 4 :, :]
        nc.sync.dma_start(
            out=sgat[:], in_=stat_rows.rearrange("c r f -> c (r f)").unsqueeze(0)
        )
        acc = sb.tile([1, 2 * H1], f32, tag="sacc")
        nc.vector.tensor_add(
            out=acc[:], in0=sgat[:, 0, : 2 * H1], in1=sgat[:, 0, 2 * H1 :]
        )
        for c in range(1, NCORES):
            nc.vector.tensor_add(out=acc[:], in0=acc[:], in1=sgat[:, c, : 2 * H1])
            nc.vector.tensor_add(out=acc[:], in0=acc[:], in1=sgat[:, c, 2 * H1 :])
        # bn1 coeffs (row orientation [1, H1])
        mu = sb.tile([1, H1], f32, tag="mu")
        nc.vector.tensor_scalar_mul(out=mu[:], in0=acc[:, :H1], scalar1=1.0 / N)
        va = sb.tile([1, H1], f32, tag="va")
        nc.vector.tensor_scalar_mul(out=va[:], in0=acc[:, H1:], scalar1=1.0 / N)
        musq = sb.tile([1, H1], f32, tag="musq")
        nc.vector.tensor_mul(out=musq[:], in0=mu[:], in1=mu[:])
        nc.vector.tensor_sub(out=va[:], in0=va[:], in1=musq[:])
        nc.vector.tensor_scalar_add(out=va[:], in0=va[:], scalar1=EPS)
        sd = sb.tile([1, H1], f32, tag="sd")
        nc.scalar.activation(out=sd[:], in_=va[:], func=AF.Sqrt)
        rs = sb.tile([1, H1], f32, tag="rs")
        nc.vector.reciprocal(out=rs[:], in_=sd[:])
        sc_r = sb.tile([1, H1], f32, tag="sc_r")
        nc.vector.tensor_mul(out=sc_r[:], in0=rs[:], in1=bng1r[:])
        sh_r = sb.tile([1, H1], f32, tag="sh_r")
        nc.vector.tensor_mul(out=sh_r[:], in0=mu[:], in1=sc_r[:])
        nc.vector.tensor_sub(out=sh_r[:], in0=bnb1r[:], in1=sh_r[:])
        # broadcast [P, H1] tiles for the folded gather affine
        scp = pp_m.tile([P, H1], f32, tag="m")
        nc.tensor.matmul(out=scp[:], lhsT=onesr[:], rhs=sc_r[:], start=True, stop=True)
        scale_bc = cst.tile([P, H1], f32, tag="scale_bc")
        nc.vector.tensor_copy(out=scale_bc[:], in_=scp[:])
        shp = pp_m.tile([P, H1], f32, tag="m")
        nc.tensor.matmul(out=shp[:], lhsT=onesr[:], rhs=sh_r[:], start=True, stop=True)
        shift_bc = cst.tile([P, H1], f32, tag="shift_bc")
        nc.vector.tensor_copy(out=shift_bc[:], in_=shp[:])
        # column coeffs [H1, 1] for the transposed h1 (root2 operand)
        sccp = pp_m.tile([H1, 1], f32, tag="m")
        nc.tensor.transpose(out=sccp[:], in_=sc_r[:], identity=ident[:1, :1])
        sc_c = sb.tile([H1, 1], f32, tag="sc_c")
        nc.vector.tensor_copy(out=sc_c[:], in_=sccp[:])
        shcp = pp_m.tile([H1, 1], f32, tag="m")
        nc.tensor.transpose(out=shcp[:], in_=sh_r[:], identity=ident[:1, :1])
        sh_c = sb.tile([H1, 1], f32, tag="sh_c")
        nc.vector.tensor_copy(out=sh_c[:], in_=shcp[:])
        # normalized h1^T per window with ones row (root2+bias2 operand)
        h1T_list = []
        for w in range(WPC):
            h1Ta = stash.tile([H1 + 1, P], bf16, tag="h1Ta")
            nc.vector.tensor_scalar(
                out=h1Ta[:H1, :], in0=h1T_raw[w][:], scalar1=sc_c[:, :1],
                scalar2=sh_c[:, :1], op0=AL.mult, op1=AL.add,
            )
            nc.gpsimd.memset(h1Ta[H1 : H1 + 1, :], 1.0)
            h1T_list.append(h1Ta)

        # ======================= layer 2 =======================
        gTp[0] = pp_g.tile([P, NG], f32, tag="gtp", name="gtpA")
        gTp.append(pp_g.tile([P, NG], f32, tag="gtpB", name="gtpB"))
        _, stp2a, stp2b = conv_layer(2, scale_bc, shift_bc, h1T_list)

        # one final AllReduce carries pooled graph features + BN2 stats
        fin_sb = sb.tile([P, NG + 2], f32, tag="fin")
        nc.vector.tensor_copy(out=fin_sb[:, :NG], in_=gTp[1][:])
        nc.vector.tensor_copy(out=fin_sb[:, NG : NG + 1], in_=stp2a[:])
        nc.vector.tensor_copy(out=fin_sb[:, NG + 1 : NG + 2], in_=stp2b[:])
        nc.sync.dma_start(out=fin_loc[:], in_=fin_sb[:])
        nc.gpsimd.collective_compute(
            "AllReduce", mybir.AluOpType.add, replica_groups=groups,
            ins=[fin_loc.opt()], outs=[fin_g.opt()],
        )
        fin = sb.tile([P, NG + 2], f32, tag="fin2")
        nc.sync.dma_start(out=fin[:], in_=fin_g[:])
        finA_rb = sb.tile([P, NG], f32, tag="finArb")
        nc.sync.dma_start(out=finA_rb[:], in_=finA_g[:])
        nc.vector.tensor_add(out=fin[:, :NG], in0=fin[:, :NG], in1=finA_rb[:])
        # bn2 coeffs (column orientation [H2, 1])
        mu2 = sb.tile([H2, 1], f32, tag="mu2")
        nc.vector.tensor_scalar_mul(out=mu2[:], in0=fin[:, NG : NG + 1], scalar1=1.0 / N)
        va2 = sb.tile([H2, 1], f32, tag="va2")
        nc.vector.tensor_scalar_mul(out=va2[:], in0=fin[:, NG + 1 : NG + 2], scalar1=1.0 / N)
        musq2 = sb.tile([H2, 1], f32, tag="musq2")
        nc.vector.tensor_mul(out=musq2[:], in0=mu2[:], in1=mu2[:])
        nc.vector.tensor_sub(out=va2[:], in0=va2[:], in1=musq2[:])
        nc.vector.tensor_scalar_add(out=va2[:], in0=va2[:], scalar1=EPS)
        sd2 = sb.tile([H2, 1], f32, tag="sd2")
        nc.scalar.activation(out=sd2[:], in_=va2[:], func=AF.Sqrt)
        rs2 = sb.tile([H2, 1], f32, tag="rs2")
        nc.vector.reciprocal(out=rs2[:], in_=sd2[:])
        sc2 = sb.tile([H2, 1], f32, tag="sc2")
        nc.vector.tensor_mul(out=sc2[:], in0=rs2[:], in1=bng2[:])
        sh2 = sb.tile([H2, 1], f32, tag="sh2")
        nc.vector.tensor_mul(out=sh2[:], in0=mu2[:], in1=sc2[:])
        nc.vector.tensor_sub(out=sh2[:], in0=bnb2[:], in1=sh2[:])
        # g = sc2 * g_raw + sh2 * gmask   (BN2 folded through the pool)
        gt = sb.tile([P, NG], f32, tag="gt")
        nc.vector.tensor_scalar_mul(out=gt[:], in0=fin[:, :NG], scalar1=sc2[:, :1])
        nc.vector.scalar_tensor_tensor(
            out=gt[:], in0=gmaskb[:], scalar=sh2[:, :1], in1=gt[:],
            op0=AL.mult, op1=AL.add,
        )

        # ======================= final MLP =======================
        l1p = pp_m.tile([H1, NG], f32, tag="m")
        nc.tensor.matmul(out=l1p[:], lhsT=l1W[:], rhs=gt[:], start=True, stop=True)
        hl = sb.tile([H1, NG], f32, tag="hl")
        nc.vector.tensor_scalar(
            out=hl[:], in0=l1p[:], scalar1=l1b[:, :1], scalar2=0.0,
            op0=AL.add, op1=AL.max,
        )
        l2p = pp_m.tile([1, NG], f32, tag="m")
        nc.tensor.matmul(out=l2p[:], lhsT=l2W[:], rhs=hl[:], start=True, stop=True)
        osb = sb.tile([1, NG], f32, tag="osb")
        nc.vector.tensor_scalar_add(out=osb[:], in0=l2p[:], scalar1=l2b[:, :1])
        nc.sync.dma_start(out=out_d[:], in_=osb[:])

    nc.compile()
    return nc


_CACHE = {}


def _get_program(T, ES, ttypes):
    key = (T, ES, ttypes)
    if key not in _CACHE:
        _CACHE[key] = build_program(T, ES, ttypes)
    return _CACHE[key]


def make_in_maps(inputs):
    pp = _preprocess(
        inputs["x"], inputs["edge_index"], inputs["edge_attr"], inputs["batch"]
    )
    w = _weights(inputs)
    bf = ml_dtypes.bfloat16
    shared = dict(
        W1a1=w["W1a1"].astype(bf), W1a2=w["W1a2"].astype(bf),
        Wp1=w["Wp1"].astype(bf), Wp2=w["Wp2"].astype(bf),
        root1a=w["root1a"].astype(bf), root2a=w["root2a"].astype(bf),
        bng1r=w["bng1r"], bnb1r=w["bnb1r"], bng2=w["bng2"], bnb2=w["bnb2"],
        l1W=w["l1W"], l1b=w["l1b"], l2W=w["l2W"], l2b=w["l2b"],
        onesP=w["onesP"], onesr=w["onesr"],
        gmaskb=np.ascontiguousarray(
            np.broadcast_to(pp["gmask"], (P, NG)).astype(np.float32)
        ),
    )
    in_maps = []
    for c in range(NCORES):
        m = dict(shared)
        m["eaT"] = np.ascontiguousarray(pp["eaT"][c].astype(bf))
        m["srchA"] = np.ascontiguousarray(pp["srchA"][c])
        m["srchB"] = np.ascontiguousarray(pp["srchB"][c])
        m["ohall"] = np.ascontiguousarray(pp["ohall"][c])
        m["ohgall"] = np.ascontiguousarray(pp["ohgall"][c])
        m["icnt"] = np.ascontiguousarray(pp["icnt"][c])
        m["xsrc"] = np.ascontiguousarray(pp["xsrc"][c])
        m["xTa"] = np.ascontiguousarray(pp["xTa"][c].astype(bf))
        m["igc"] = np.ascontiguousarray(pp["igc"][c])
        m["vmask"] = np.ascontiguousarray(pp["vmask"][c])
        in_maps.append(m)
    return in_maps, pp["T"], pp["ES"], pp["ttypes"]


def _run(inputs, trace=False):
    in_maps, T, ES, ttypes = make_in_maps(inputs)
    nc = _get_program(T, ES, ttypes)
    res = run_bass_kernel_spmd(
        nc, in_maps, core_ids=list(range(NCORES)), trace=trace
    )
    out = np.asarray(res.results[0]["out"][0], dtype=np.float32)
    return out, res


def kernel(**inputs):
    return _run(inputs)[0]
